# revision 39
# baseline (speedup 1.0000x reference)
"""Trainium2 Bass kernel: Convpass adapter with hypernet-generated 3x3 conv.

Per core (data-parallel over batch, 8 images/core):
  hypernet: conv_w = emb @ w_hyper + b_hyper, via the diag-window matmul
            trick with both o-halves packed on 128 partitions (64 matmuls
            of N=288). The 4.7MB bf16 w_hyper streams in 8 DMA chunks with
            matmuls chasing the chunks; w_conv2 is built in two 32-row
            groups overlapped with the stream.
  down:     xT[128c,4k,784] @ [w_down|w_down] -> psum [128, 392] per half
            (x arrives pre-transposed bf16 from the host; psum rows 0-63 ==
            rows 64-127 so the conv can pack 2 taps)
  gelu1:    quickgelu(x+b) as a single scalar-engine Gelu_apprx_sigmoid
            activation from psum, written twice: rows 0-63 at col+1 (dx=0
            taps), rows 64-127 at col (dx=1 taps) of a padded buffer
  conv:     3x3 as 3 K=128 matmuls (dx=0,1 packed) + 3 K=64 (dx=2)
  gelu2:    quickgelu(scale*y) as one activation per half into y_act
  up:       out^T[128c,392] = w_up65[:,cslice].T @ y_act  (stationary w_up,
            ones-row fused bias); stored transposed bf16, host untransposes.

All small constants (w_down2 dup, w_up65, the hypernet lhsT window tensor,
the rearranged conv bias) are assembled host-side in bf16 and loaded as one
contiguous tensor over the fast hardware DGE ring — the software-DGE cast
path measured 20+us of serialized small packets and gated the first matmul.
The image loop is software-pipelined for the in-order PE queue: tensor
order is conv(i), down(i+1), up(i), with gelu1(i+1) issued right after
down(i+1) so its activations land early in the scalar queue.
"""

import os

import numpy as np
import ml_dtypes

import concourse.bass as bass
import concourse.mybir as mybir
import concourse.tile as tile
from concourse import bacc
from concourse.bass_utils import run_bass_kernel_spmd

# Problem shapes (hardcoded per contract).
B, H, W, C = 64, 28, 28, 512
DIM, EMB = 64, 64
NCORES = 8
B_LOC = B // NCORES            # 8 images per core
PIX = H * W                    # 784 pixels per image
PW = W + 2                     # 30 padded width
PAD = PW * (H + 2)             # 900 padded pixels per image
RH = 2                         # row-halves per image
RROWS = H // RH                # 14 rows per half
NHALF = RROWS * W              # 392 pixels per half-tile
KCH = C // 128                 # 4 contraction chunks of 128 channels
JTOT = DIM * DIM * 9           # 36864 hypernet outputs
NHYP = 32 * 9                  # 288 = free size of packed hypernet matmuls

NCHUNK = 16                    # w_hyper streaming DMA chunks
ICH = DIM // NCHUNK            # 8 i-rows per DMA chunk
IGRP = 32                      # i-rows per compute/build group (32-aligned)

# packed-const column offsets (bf16 [128, CPACK_W])
CP_WDOWN = 0                   # [128, 512]  w_down duplicated, (k m) layout
CP_WUP = 512                   # [65, 512]   w_up with bias row 64
CP_T2 = 1024                   # [128, 192]  hypernet lhsT window tensor
CP_BPREP = 1216                # [128, 288]  conv bias, psum-row layout
CPACK_W = 1504

F32 = mybir.dt.float32
BF16 = mybir.dt.bfloat16
GELU_A = 1.702
# CoreSim doesn't implement Gelu_apprx_sigmoid; substitute Sigmoid for
# structure-only sim runs (numerics then checked on HW via --randup).
ACT_QGELU = (
    mybir.ActivationFunctionType.Sigmoid
    if os.environ.get("KERNEL_DEBUG_SIM_ACT") == "1"
    else mybir.ActivationFunctionType.Gelu_apprx_sigmoid
)

_CACHE = {}


def build_kernel():
    if "nc" in _CACHE:
        return _CACHE["nc"]

    nc = bacc.Bacc("TRN2", target_bir_lowering=False, debug=False)

    x_d = nc.dram_tensor("x", [B_LOC, 128, KCH * PIX], BF16, kind="ExternalInput")
    cpk_d = nc.dram_tensor("cpack", [128, CPACK_W], BF16, kind="ExternalInput")
    cf_d = nc.dram_tensor("cf32", [128, 2], F32, kind="ExternalInput")
    # host-packed hypernet: [128, i, ol, t]; rows 0-63 = o<32, 64-127 = o>=32
    wh_d = nc.dram_tensor("w_hyper", [128, DIM * NHYP], BF16, kind="ExternalInput")
    out_d = nc.dram_tensor("out", [B_LOC, 128, KCH * PIX], BF16, kind="ExternalOutput")

    with tile.TileContext(nc) as tc:
        with tc.tile_pool(name="consts", bufs=1) as consts:
            # ---- constants: 6 contiguous HWDGE loads ----
            w_down2 = consts.tile([128, KCH, 128], BF16)
            nc.sync.dma_start(
                w_down2[:].rearrange("p k m -> p (k m)"),
                cpk_d[:, CP_WDOWN : CP_WDOWN + 512],
            )
            w_up65 = consts.tile([DIM + 1, C], BF16)
            nc.sync.dma_start(w_up65[:], cpk_d[0 : DIM + 1, CP_WUP : CP_WUP + 512])
            t2 = consts.tile([128, 192], BF16)
            nc.sync.dma_start(t2[:], cpk_d[:, CP_T2 : CP_T2 + 192])
            b_prep2 = consts.tile([128, NHYP], BF16)
            nc.sync.dma_start(b_prep2[:], cpk_d[:, CP_BPREP : CP_BPREP + NHYP])
            b_down2 = consts.tile([128, 1], F32)
            nc.sync.dma_start(b_down2[:], cf_d[:, 0:1])
            scale_sb = consts.tile([DIM, 1], F32)
            nc.sync.dma_start(scale_sb[:], cf_d[0:DIM, 1:2])

            # rows 0-63: W[i, o*9+t]; rows 64-127: same shifted by one tap so
            # a K=128 conv matmul contracts taps (dy,0) and (dy,1) at once.
            w_conv2 = consts.tile([128, DIM * 9], BF16)
            # w_conv6: rows 64-127 shifted by SIX taps so a K=128 matmul
            # contracts the vertical tap pair (0,dx)+(2,dx) at once.
            w_conv6 = consts.tile([128, DIM * 9], BF16)

            # ---- main pools ----
            with (
                tc.tile_pool(name="whpool", bufs=1) as whpool,
                tc.tile_pool(name="xin", bufs=4) as xin,
                tc.tile_pool(name="xact", bufs=4) as xactp,
                tc.tile_pool(name="xact2", bufs=4) as xact2p,
                tc.tile_pool(name="yact", bufs=3) as yactp,
                tc.tile_pool(name="tmp", bufs=6) as tmpp,
                tc.tile_pool(name="outs", bufs=2) as outsp,
                tc.tile_pool(name="ps_s", bufs=4, space="PSUM") as ps_sp,
                tc.tile_pool(name="ps_c", bufs=2, space="PSUM") as ps_cp,
                tc.tile_pool(name="ps_u", bufs=2, space="PSUM") as ps_up,
            ):
                # ---- prologue ----
                # one tile PER w_hyper chunk: tile-granular dependency
                # tracking would stall the first hypernet matmul until the
                # last chunk landed if this were a single tile
                wh_chunks = []
                for q in range(NCHUNK):
                    cw = ICH * NHYP
                    t = whpool.tile([128, cw], BF16, tag=f"wh{q}")
                    eng = nc.scalar if q % 2 == 0 else nc.sync
                    eng.dma_start(t[:], wh_d[:, q * cw : (q + 1) * cw])
                    wh_chunks.append(t)

                def load_x(img):
                    xT = xin.tile([128, KCH, PIX], BF16, tag="x", name=f"x{img}")
                    nc.sync.dma_start(
                        xT[:].rearrange("p k n -> p (k n)"), x_d[img]
                    )
                    return xT

                xTs = [load_x(0)]

                def make_xacts(img):
                    x_act = xactp.tile([128, PAD], BF16, tag="xa", name=f"xa{img}")
                    nc.gpsimd.memset(x_act[:], 0.0)
                    x_act2 = xact2p.tile([128, PAD], BF16, tag="xb", name=f"xb{img}")
                    nc.gpsimd.memset(x_act2[:], 0.0)
                    return (x_act, x_act2)

                xacts = [make_xacts(0), make_xacts(1)]

                def wh_slice(il):
                    """rhs [128, 288] for hypernet row il, from its chunk."""
                    t = wh_chunks[il // ICH]
                    j = il % ICH
                    return t[:, j * NHYP : (j + 1) * NHYP]

                def down(img, xT):
                    """down-proj matmuls -> 2 psum tiles [128, 392]"""
                    ps_ds = [
                        ps_sp.tile([128, NHALF], F32, tag="pss", name=f"psd{img}_{rh}")
                        for rh in range(RH)
                    ]
                    for k in range(KCH):
                        for rh in range(RH):
                            nc.tensor.matmul(
                                ps_ds[rh][:],
                                w_down2[:, k, :],
                                xT[:, k, rh * NHALF : (rh + 1) * NHALF],
                                start=(k == 0),
                                stop=(k == KCH - 1),
                            )
                    return ps_ds

                def gelu1(img, ps_ds, x_act, x_act2):
                    """quickgelu(x+b) = Gelu_apprx_sigmoid(1.0*x + b) from psum
                    into two padded buffers: A top = act at (r+1,c+1), A
                    bottom = (r+1,c) [1-col shift, packs taps (1,0)+(1,1)];
                    A2 top = same as A top, A2 bottom = (r-1,c+1) [2-row
                    shift, packs taps (0,dx)+(2,dx)]."""
                    x_act_v = x_act[:].rearrange("d (r c) -> d r c", c=PW)
                    x2_v = x_act2[:].rearrange("d (r c) -> d r c", c=PW)
                    for rh in range(RH):
                        ps_d = ps_ds[rh]
                        ps_v = ps_d[:].rearrange("d (r c) -> d r c", c=W)
                        rows = slice(1 + rh * RROWS, 1 + (rh + 1) * RROWS)
                        nc.scalar.activation(
                            x_act_v[:DIM, rows, 1 : 1 + W],
                            ps_v[:DIM],
                            ACT_QGELU,
                            bias=b_down2[:DIM],
                            scale=1.0,
                        )
                        nc.scalar.activation(
                            x_act_v[DIM:, rows, 0:W],
                            ps_v[DIM:],
                            ACT_QGELU,
                            bias=b_down2[DIM:],
                            scale=1.0,
                        )
                        # A2 top duplicates A top (SBUF->SBUF, off scalar)
                        nc.vector.tensor_copy(
                            x2_v[:DIM, rows, 1 : 1 + W],
                            x_act_v[:DIM, rows, 1 : 1 + W],
                        )
                        # A2 bottom holds the same values two padded rows up
                        # and on partitions 64-127: a row/partition-shifting
                        # SBUF->SBUF DMA of A top (act row 0 would land at
                        # padded row -1, which is never read, so skip it)
                        if rh == 0:
                            nc.sync.dma_start(
                                x2_v[DIM:, 0 : RROWS - 1, 1 : 1 + W],
                                x_act_v[:DIM, 2 : 1 + RROWS, 1 : 1 + W],
                            )
                        else:
                            nc.sync.dma_start(
                                x2_v[DIM:, RROWS - 1 : 2 * RROWS - 1, 1 : 1 + W],
                                x_act_v[:DIM, 1 + RROWS : 1 + 2 * RROWS, 1 : 1 + W],
                            )
                    return (x_act_v, x2_v)

                # Prologue: downs go AFTER the first hypernet group in the
                # in-order PE queue (a stalled down matmul ahead of the
                # hypernet delays conv0 behind it).
                downed = []

                def issue_down(img):
                    ps = down(img, xTs[img])
                    xa = gelu1(img, ps, *xacts[img])
                    downed.append((ps, xa))

                # full hypernet: matmuls chase the streaming w_hyper DMA
                # chunk by chunk (region-level deps); psum/build work in two
                # 32-row groups (engine partition slices need 32 alignment).
                # Group g's psum rows [32g,32g+32) = W[i, o<32], rows
                # [64+32g, ..) = W[i, o>=32].
                for g in range(DIM // IGRP):
                    ps_q = ps_up.tile([128, NHYP], F32, tag="psu", name=f"hyp{g}")
                    for il in range(g * IGRP, (g + 1) * IGRP):
                        nc.tensor.matmul(
                            ps_q[:],
                            t2[:, 64 - il : 192 - il],
                            wh_slice(il),
                            start=(il % IGRP == 0),
                            stop=(il % IGRP == IGRP - 1),
                        )
                    if g == 0:
                        # two images' downs fill the w_hyper chunk-wait gaps
                        # between the hypernet groups
                        issue_down(0)
                        xTs.append(load_x(1))
                        issue_down(1)
                        xTs.append(load_x(2))
                    rt = slice(g * IGRP, (g + 1) * IGRP)
                    rb = slice(DIM + g * IGRP, DIM + (g + 1) * IGRP)
                    nc.vector.tensor_tensor(
                        w_conv2[rt, :NHYP], ps_q[rt, :], b_prep2[rt, :],
                        mybir.AluOpType.add,
                    )
                    t_b = tmpp.tile([128, NHYP], BF16, tag="t")
                    nc.vector.tensor_tensor(
                        t_b[rb, :], ps_q[rb, :], b_prep2[rb, :],
                        mybir.AluOpType.add,
                    )
                    nc.scalar.dma_start(w_conv2[rt, NHYP:], t_b[rb, :])
                    # bottom half rows = top rows shifted by one tap
                    nc.scalar.dma_start(
                        w_conv2[rb, : DIM * 9 - 1], w_conv2[rt, 1 : DIM * 9]
                    )
                    # w_conv6: same top; bottom shifted by six taps
                    nc.vector.tensor_copy(w_conv6[rt, :], w_conv2[rt, :])
                    nc.scalar.dma_start(
                        w_conv6[rb, : DIM * 9 - 6], w_conv2[rt, 6 : DIM * 9]
                    )
                nc.vector.memset(w_conv2[DIM:, DIM * 9 - 1 :], 0.0)
                nc.vector.memset(w_conv6[DIM:, DIM * 9 - 6 :], 0.0)
                w_conv_v = w_conv2[:].rearrange("i (o t) -> i o t", t=9)
                w_conv6_v = w_conv6[:].rearrange("i (o t) -> i o t", t=9)

                for img in range(B_LOC):
                    xact_cur, xact2_cur = downed[img][1]
                    # conv, 5 matmuls per half: 3 vertical pairs
                    # (0,dx)+(2,dx) on A2/w_conv6, the pair (1,0)+(1,1) on
                    # A/w_conv2, and the lone (1,2) tap at K=64
                    ps_cs = []
                    for rh in range(RH):
                        ps_c = ps_cp.tile(
                            [DIM, NHALF], F32, tag="psc", name=f"psc{img}_{rh}"
                        )
                        for dx in range(3):
                            src = xact2_cur[
                                :, rh * RROWS : rh * RROWS + RROWS, dx : dx + W
                            ]
                            nc.tensor.matmul(
                                ps_c[:],
                                w_conv6_v[:, :, dx],
                                src,
                                start=(dx == 0),
                                stop=False,
                            )
                        nc.tensor.matmul(
                            ps_c[:],
                            w_conv_v[:, :, 3],
                            xact_cur[
                                :, rh * RROWS + 1 : rh * RROWS + 1 + RROWS, 0:W
                            ],
                            start=False,
                            stop=False,
                        )
                        nc.tensor.matmul(
                            ps_c[:],
                            w_conv_v[:DIM, :, 5],
                            xact_cur[
                                :DIM,
                                rh * RROWS + 1 : rh * RROWS + 1 + RROWS,
                                2 : 2 + W,
                            ],
                            start=False,
                            stop=True,
                        )
                        ps_cs.append(ps_c)

                    # pipelined: issue image img+2's load+down+gelu1 now —
                    # the gelu1 activations land early in the scalar queue so
                    # conv(img+2) never waits on them, and the down matmuls
                    # keep the PE busy while gelu2 below drains conv psum
                    if img + 2 < B_LOC:
                        if img + 3 < B_LOC:
                            xTs.append(load_x(img + 3))
                        xacts.append(make_xacts(img + 2))
                        issue_down(img + 2)

                    # gelu2: quickgelu(scale*y) = Gelu_apprx_sigmoid(scale*y)
                    # straight from psum into y_act (ones row fuses up bias)
                    y_act = yactp.tile([DIM + 1, PIX], BF16, tag="ya")
                    nc.vector.memset(y_act[DIM : DIM + 1, :], 1.0)
                    for rh in range(RH):
                        nc.scalar.activation(
                            y_act[:DIM, rh * NHALF : (rh + 1) * NHALF],
                            ps_cs[rh][:],
                            ACT_QGELU,
                            bias=0.0,
                            scale=scale_sb[:],
                        )

                    # up-proj + bias, transposed: out^T[c,pix] per c-chunk;
                    # the output DMA goes out in two halves so the last
                    # image's store starts before its second half is copied
                    o_sb = outsp.tile([128, KCH, PIX], BF16, tag="o")
                    for kc in range(KCH):
                        for rh in range(RH):
                            ps_u = ps_up.tile([128, NHALF], F32, tag="psu")
                            nc.tensor.matmul(
                                ps_u[:],
                                w_up65[:, kc * 128 : (kc + 1) * 128],
                                y_act[:, rh * NHALF : (rh + 1) * NHALF],
                                start=True,
                                stop=True,
                            )
                            dst = o_sb[:, kc, rh * NHALF : (rh + 1) * NHALF]
                            j = kc * RH + rh
                            if j == 3:
                                nc.scalar.copy(dst, ps_u[:])
                            else:
                                nc.vector.tensor_copy(dst, ps_u[:])
                        if kc == 1:
                            nc.scalar.dma_start(
                                out_d[img][:, : 2 * PIX],
                                o_sb[:, 0:2, :].rearrange("p k n -> p (k n)"),
                            )
                    nc.scalar.dma_start(
                        out_d[img][:, 2 * PIX :],
                        o_sb[:, 2:4, :].rearrange("p k n -> p (k n)"),
                    )

    nc.compile()
    _CACHE["nc"] = nc
    return nc


def _make_in_maps(inputs):
    bf16 = ml_dtypes.bfloat16
    x = np.ascontiguousarray(inputs["x"], dtype=np.float32)

    # ---- packed bf16 consts ----
    cpk = np.zeros((128, CPACK_W), dtype=bf16)
    wd = np.asarray(inputs["w_down"], np.float32).astype(bf16)
    t = wd.reshape(KCH, 128, DIM).transpose(1, 0, 2)       # [p, k, d]
    cpk[:, CP_WDOWN : CP_WDOWN + 512] = np.concatenate(
        [t, t], axis=2
    ).reshape(128, 512)
    cpk[0:DIM, CP_WUP : CP_WUP + 512] = np.asarray(
        inputs["w_up"], np.float32
    ).astype(bf16)
    cpk[DIM, CP_WUP : CP_WUP + 512] = np.asarray(
        inputs["b_up"], np.float32
    ).astype(bf16)
    emb = np.asarray(inputs["layer_emb"], np.float32).astype(bf16)
    cpk[0:EMB, CP_T2 + 64] = emb
    cpk[EMB:128, CP_T2 + 128] = emb
    bh = np.asarray(inputs["b_hyper"], np.float32).reshape(DIM, DIM, 9)
    b_ot = bh.transpose(1, 0, 2).astype(bf16)              # [i, o, t]
    cpk[0:DIM, CP_BPREP : CP_BPREP + NHYP] = b_ot[:, :32].reshape(DIM, NHYP)
    cpk[DIM:, CP_BPREP : CP_BPREP + NHYP] = b_ot[:, 32:].reshape(DIM, NHYP)

    cf = np.zeros((128, 2), np.float32)
    bd = np.asarray(inputs["b_down"], np.float32)
    cf[0:DIM, 0] = bd
    cf[DIM:, 0] = bd
    cf[0:DIM, 1] = np.asarray(inputs["scale"], np.float32)

    # ---- packed hypernet: [128, i, ol, t]; rows 0-63 = o<32 block ----
    wh = np.asarray(inputs["w_hyper"], np.float32).astype(bf16)
    wh = wh.reshape(EMB, DIM, DIM, 9)                      # [e, o, i, t]
    top = wh[:, :32].transpose(0, 2, 1, 3)                 # [e, i, ol, t]
    bot = wh[:, 32:].transpose(0, 2, 1, 3)
    whp = np.ascontiguousarray(
        np.concatenate([top, bot], axis=0).reshape(128, DIM * NHYP)
    )

    shared = {"cpack": cpk, "cf32": cf, "w_hyper": whp}
    in_maps = []
    for c in range(NCORES):
        xc = x[c * B_LOC : (c + 1) * B_LOC].reshape(B_LOC, PIX, KCH, 128)
        xt = np.ascontiguousarray(xc.transpose(0, 3, 2, 1)).astype(bf16)
        in_maps.append({"x": xt.reshape(B_LOC, 128, KCH * PIX), **shared})
    return in_maps


def _untranspose_out(res):
    outs = []
    for c in range(NCORES):
        o = np.asarray(res.results[c]["out"]).reshape(B_LOC, 128, KCH, PIX)
        o = o.transpose(0, 3, 2, 1).astype(np.float32)  # [img, pix, kc, p]
        outs.append(o.reshape(B_LOC, H, W, C))
    return np.concatenate(outs, axis=0)


def kernel(**inputs) -> np.ndarray:
    nc = build_kernel()
    in_maps = _make_in_maps(inputs)
    res = run_bass_kernel_spmd(nc, in_maps, core_ids=list(range(NCORES)))
    return _untranspose_out(res)


def run_traced(inputs, **kw):
    """For test.py: run with tracing to get HW exec time."""
    nc = build_kernel()
    in_maps = _make_in_maps(inputs)
    return run_bass_kernel_spmd(
        nc, in_maps, core_ids=list(range(NCORES)), trace=True, **kw
    )
